# revision 1
# baseline (speedup 1.0000x reference)
"""Trainium2 Bass kernel for deformable orientation sampling (DeoLayer).

Math:
  out[b,c,o,h,w] = (1-w1)*x[b,c,i0,h,w] + w1*x[b,c,i1,h,w]
  where p = o + offset[b,g(c),o,h,w], i0 = floor(p) mod O, i1 = (i0+1) mod O,
  w1 = frac(offset), O = 8 orientations, G = 8 groups (32 channels each).

Reformulated as a dense 8-term cyclic weighted sum with "periodized hat"
coefficients (exact: non-contributing terms are exactly 0, so the fp32 sum
equals the 2-term lerp up to ~1 ulp of the weights):
  out[...,o,hw] = sum_{s=0..7} C_s[g,o,hw] * x[...,(o+s)%8,hw]
  C_s = sum_k relu(1 - |offset - (s + 8k)|)   (hats have disjoint support)

Distribution: pure data parallel, batch b -> core b (B=8, 8 cores, no
communication).

Per-core layout: SBUF partition p = g*16 + v (g in [0,8), v in [0,16)),
hw = v*256 + u, u in [0,256). Free dims per partition: (c, o/j, u).
C_s is shared by the 32 channels of a group; engine operands broadcast it
along the c free-dim with a stride-0 AP dimension (no physical replication).
The cyclic roll (o+s)%8 is two free-dim j-subranges (no partition moves).

Engines: DVE and GPSIMD split the u-range of the multiply/accumulate
passes (fp32 tensor_tensor never contends for the shared SBUF port); ACT
evaluates the hat relu()s; DMAs are HWDGE on the SP and ACT rings.
"""

import os
import sys

import numpy as np

if "/opt/trn_rl_repo" not in sys.path:
    sys.path.insert(0, "/opt/trn_rl_repo")

# Problem constants (hardcoded per harness contract).
B, C, O, H, W = 8, 256, 8, 64, 64
G = 8
CPG = C // G          # 32 channels per group
HW = H * W            # 4096
NCORES = 8
VPART = 16            # hw-high slices per group on partitions: p = g*16 + v
UFULL = HW // VPART   # 256 hw elements per partition
CP = 2                # channels per c-pass
NCPASS = CPG // CP    # 16 passes
# Static hat centers valid for |offset| < OFF_BOUND (13 hats total).
OFF_BOUND = 5.999
STATIC_CENTERS = [[0], [1], [-6, 2], [-5, 3], [-4, 4], [-3, 5], [-2, 6], [-1]]
# u-split between DVE [0, UD) and GPSIMD [UD, UFULL).
# DVE ~123G elem-ops/s, GPSIMD ~59G -> ~2:1.
UD = int(os.environ.get("BASS_DEO_UD", "172"))

_PROGRAM_CACHE = {}


def _centers_for_bound(maxa: float):
    kmax = int(maxa) // 8 + 2
    out = []
    for s in range(O):
        cs = [s + 8 * k for k in range(-kmax, kmax + 1)
              if (s + 8 * k - 1 < maxa) and (s + 8 * k + 1 > -maxa)]
        out.append(cs)
    return out


def _build_program(centers):
    import concourse.bass as bass
    import concourse.tile as tile
    from concourse import bacc, mybir

    assert centers[0], "s=0 must have a hat center (ost init depends on it)"
    f32 = mybir.dt.float32
    # Bacc (not Bass): its compile() runs generate_event_semaphores(), which
    # splits multi-sem sync waits — TRN2 instructions carry at most one.
    nc = bacc.Bacc("TRN2", target_bir_lowering=False, debug=False)
    x_d = nc.declare_dram_parameter("x", [C, O, HW], f32, isOutput=False)
    off_d = nc.declare_dram_parameter("offset", [G, O, HW], f32, isOutput=False)
    out_d = nc.declare_dram_parameter("out", [C, O, HW], f32, isOutput=True)

    # DRAM views: [g, v, <rest>] with v = hw-high (256-element runs stay
    # contiguous as the DMA descriptor payload). Stores iterate (v, o, u) so
    # the out AP leads with the 16-wide dim (keeps per-iteration bytes low).
    x_r = x_d[:].rearrange("(g c) j (v u) -> g c j v u", g=G, v=VPART)
    out_r = out_d[:].rearrange("c o (v u) -> c v o u", v=VPART)
    off_r = off_d[:].rearrange("g o (v u) -> g o v u", v=VPART)

    engine_slices = []
    if UD > 0:
        engine_slices.append(("v", 0, UD))
    if UD < UFULL:
        engine_slices.append(("g", UD, UFULL))

    with tile.TileContext(nc) as tc:
        with (
            tc.tile_pool(name="offp", bufs=1) as offp,
            tc.tile_pool(name="coefp", bufs=1) as coefp,
            tc.tile_pool(name="hatp", bufs=1) as hatp,
            tc.tile_pool(name="xp", bufs=2) as xp,
            tc.tile_pool(name="op", bufs=2) as op,
            tc.tile_pool(name="tp", bufs=1) as tp,
        ):
            offs = offp.tile([128, O, UFULL], f32)
            for o in range(O):
                # DRAM [8g, 16v, 256u] -> SBUF [128p, 256]; 1 KiB descriptors.
                # Split across both rings so the hat chain starts sooner.
                deng = nc.scalar if o % 2 == 0 else nc.sync
                deng.dma_start(out=offs[:, o, :], in_=off_r[:, o])

            # Per-partition bias columns holding -center for each hat.
            all_cens = sorted({c for cs in centers for c in cs})
            cen_col = {c: i for i, c in enumerate(all_cens)}
            bias_t = offp.tile([128, len(all_cens)], f32)
            for c, i in cen_col.items():
                nc.vector.memset(bias_t[:, i:i + 1], float(-c))

            # --- coefficient planes C_s (built once, full u range) -------
            coef = coefp.tile([128, O, O, UFULL], f32)  # [p, s, o, u] 64 KiB
            for s in range(O):
                first = True
                for cen in centers[s]:
                    bcol = bias_t[:, cen_col[cen]:cen_col[cen] + 1]
                    if first:
                        zt = hatp.tile([128, O, UFULL], f32, tag="zt")
                        # z = |offset - cen| on ACT (Abs with bias=-cen)
                        nc.scalar.activation(
                            out=zt[:], in_=offs[:],
                            func=mybir.ActivationFunctionType.Abs,
                            bias=bcol, scale=1.0)
                        # C_s = relu(1 - z) on ACT
                        nc.scalar.activation(
                            out=coef[:, s], in_=zt[:],
                            func=mybir.ActivationFunctionType.Relu,
                            bias=1.0, scale=-1.0)
                        first = False
                    else:
                        zt2 = hatp.tile([128, O, UFULL], f32, tag="zt2")
                        ht = hatp.tile([128, O, UFULL], f32, tag="ht")
                        nc.scalar.activation(
                            out=zt2[:], in_=offs[:],
                            func=mybir.ActivationFunctionType.Abs,
                            bias=bcol, scale=1.0)
                        nc.scalar.activation(
                            out=ht[:], in_=zt2[:],
                            func=mybir.ActivationFunctionType.Relu,
                            bias=1.0, scale=-1.0)
                        # disjoint supports -> add is exact; on GPSIMD to
                        # keep the DVE stream free for the main multiplies
                        nc.gpsimd.tensor_add(
                            out=coef[:, s], in0=coef[:, s], in1=ht[:])

            # --- main loop: c-passes of 2 channels ------------------------
            # xs carries a j-extended copy (j' = j mod 8 for j' in [0,15)) so
            # every roll (o+s)%8 is one contiguous j-slice [s, s+8) — no
            # free-dim splits, one mult per term per engine.
            JX = 2 * O - 1
            plan = [CP] * NCPASS  # channel count per pass
            c0 = 0
            for pi, cp in enumerate(plan):
                tail_pass = pi == len(plan) - 1
                xs = xp.tile([128, cp, JX, UFULL], f32, tag="xs")
                for cc in range(cp):
                    for j in range(O):
                        nc.sync.dma_start(
                            out=xs[:, cc, j, :],
                            in_=x_r[:, c0 + cc, j])
                for cc in range(cp):
                    # j-extension: first two tiles on DVE (which idles until
                    # the first coefficient plane lands, and this keeps ACT's
                    # early hat chain unbroken); later tiles on ACT.
                    if pi < 2:
                        nc.vector.tensor_copy(
                            out=xs[:, cc, O:JX, :], in_=xs[:, cc, 0:O - 1, :])
                    else:
                        nc.scalar.copy(
                            out=xs[:, cc, O:JX, :], in_=xs[:, cc, 0:O - 1, :])
                xsT = xs[:].transpose([0, 2, 1, 3])  # [128, j', c, u]
                ost = op.tile([128, O, cp, UFULL], f32, tag="ost")

                # The last pass computes in two u-rounds so the first half's
                # stores drain while the second half still computes ([0, 128)
                # lies entirely inside the DVE u-slice when UD > 128).
                uh2 = UFULL // 2
                if tail_pass and UD > uh2:
                    rounds = [
                        (0, uh2, [("v", 0, uh2)]),
                        (uh2, UFULL, [("v", uh2, UD), ("g", UD, UFULL)]),
                    ]
                else:
                    rounds = [(0, UFULL, engine_slices)]

                for r0, r1, eslices in rounds:
                    for ename, u0, u1 in eslices:
                        eng = nc.vector if ename == "v" else nc.gpsimd
                        ul = u1 - u0
                        if ul <= 0:
                            continue
                        tmp = tp.tile([128, O, cp, ul], f32, tag=f"tmp{ename}")
                        for s in range(O):
                            # terms with no hat center are exactly zero: skip.
                            # s=0 always has center 0 so ost is always inited.
                            if not centers[s]:
                                continue
                            dest, doff = (ost, u0) if s == 0 else (tmp, 0)
                            cb = (coef[:, s, :, u0:u1]
                                  .unsqueeze(2)
                                  .to_broadcast([128, O, cp, ul]))
                            eng.tensor_mul(
                                out=dest[:, :, :, doff:doff + ul],
                                in0=xsT[:, s:s + O, :, u0:u1],
                                in1=cb)
                            if s > 0:
                                eng.tensor_add(
                                    out=ost[:, :, :, u0:u1],
                                    in0=ost[:, :, :, u0:u1],
                                    in1=tmp[:])

                    for g in range(G):
                        for cc in range(cp):
                            # stores split across HWDGE rings; both sides
                            # iterate (v, o, u) so the out AP leads with the
                            # v=16 dim. The tail rounds use 3 rings (POOL's
                            # SWDGE ring is idle by the end of the kernel).
                            cg = g * CPG + c0 + cc
                            if tail_pass:
                                rings = [nc.scalar, nc.sync, nc.gpsimd]
                                deng = rings[(g * cp + cc) % 3]
                            else:
                                deng = nc.scalar if g % 2 == 0 else nc.sync
                            deng.dma_start(
                                out=out_r[cg][:, :, r0:r1],
                                in_=ost[g * VPART:(g + 1) * VPART,
                                        :, cc, r0:r1])
                c0 += cp
    return nc


def _get_program(centers):
    key = tuple(tuple(c) for c in centers)
    prog = _PROGRAM_CACHE.get(key)
    if prog is None:
        prog = _build_program(centers)
        # Bacc.finalize() runs compile(): register allocation + splitting of
        # multi-sem sync waits (TRN2 allows one wait per instruction).
        # run_bass_via_pjrt does not finalize prebuilt modules itself.
        prog.finalize()
        _PROGRAM_CACHE[key] = prog
    return prog


_LAST_RESULTS = None  # BassKernelResults of the most recent kernel() call


def kernel(x: np.ndarray, offset: np.ndarray) -> np.ndarray:
    global _LAST_RESULTS
    from concourse.bass_utils import run_bass_kernel_spmd

    x = np.ascontiguousarray(np.asarray(x, dtype=np.float32))
    offset = np.ascontiguousarray(np.asarray(offset, dtype=np.float32))
    assert x.shape == (B, C, O, H, W) and offset.shape == (B, G, O, H, W)

    maxa = float(np.abs(offset).max())
    centers = (STATIC_CENTERS if maxa < OFF_BOUND
               else _centers_for_bound(maxa + 1e-3))
    nc = _get_program(centers)

    xs = x.reshape(B, C, O, HW)
    offs = offset.reshape(B, G, O, HW)
    in_maps = [{"x": xs[b], "offset": offs[b]} for b in range(NCORES)]
    trace = bool(int(os.environ.get("BASS_DEO_TRACE", "0")))
    kw = {}
    if trace:
        kw["trace"] = True
        tdir = os.environ.get("BASS_DEO_TRACE_DIR")
        if tdir:
            kw["tmpdir"] = tdir
    br = run_bass_kernel_spmd(nc, in_maps, list(range(NCORES)), **kw)
    _LAST_RESULTS = br
    out = np.stack([br.results[b]["out"] for b in range(NCORES)])
    return out.reshape(B, C, O, H, W)


if __name__ == "__main__":
    xs = np.load("/tmp/x.npy")
    offs = np.load("/tmp/off.npy")
    got = kernel(xs, offs)
    exp = np.load("/tmp/expected.npy")
    d = np.abs(got - exp)
    print("absmax:", d.max(), "rel:", d.max() / np.abs(exp).max())



# revision 69
# speedup vs baseline: 2.7902x; 2.7902x over previous
"""Trainium2 Bass kernel for deformable orientation sampling (DeoLayer).

Math:
  out[b,c,o,h,w] = (1-w1)*x[b,c,i0,h,w] + w1*x[b,c,i1,h,w]
  where p = o + offset[b,g(c),o,h,w], i0 = floor(p) mod O, i1 = (i0+1) mod O,
  w1 = frac(offset), O = 8 orientations, G = 8 groups (32 channels each).

Reformulated as a dense 8-term cyclic weighted sum with "periodized hat"
coefficients (non-contributing terms are exactly 0):
  out[...,o,hw] = sum_{s=0..7} C_s[g,o,hw] * x[...,(o+s)%8,hw]
  C_s = sum_k relu(1 - |offset - (s + 8k)|)   (hats have disjoint support)

Distribution: pure data parallel, batch b -> core b (8 cores, no comms).

Schedule (built for CoreSim's v1 cost model: each engine is one serial
timeline that also executes its own DMAs; a DMA costs the out-AP's
after-first-dim bytes * 0.386ns/B (2x under 512B runs, 500ns floor); Pool
runs tensor ops at 0.833ns/elem; DVE gets 2x on fp16 tensor_tensor and 4x
on fp16 tensor_scalar; PE matmuls cost out-free-rows * 0.42ns for fp16
moving data, with free weight switches; cross-engine syncs are positional
per-engine counters, so per-engine emission order is execution order):

  * SBUF partition p = pos*16 + hw//256, u = hw%256, where pos is the
    orientation (j on inputs, o on outputs).  The (pos, hw-chunk) dims
    merge with u into a single 128-long stride-256 DRAM dim, so every
    x/out transfer is a [128, 1KB] DMA at the 500ns floor: 16 loads + 16
    stores per 2-channel pass, split across the SP and ACT queues.
  * All on-chip math fp16 (tolerance 2e-2, actual err ~2e-3).
  * The cyclic roll j -> o=(j-s)%8 is a partition-block rotation done by
    the otherwise-idle PE: per term a permutation-matrix matmul
    accumulates the product tile into PSUM (fp32).  s=0 needs no
    rotation: Pool adds its product during PSUM evacuation.
  * Products t_s = C_s * x as 16 (term, group-half) units, one
    instruction each (no roll slicing), split DVE / Pool (stride-0
    c-broadcast of the group-shared C_s).
  * Coefficient planes are built in o-partition layout (DVE tensor_scalar
    hats at 4x for s<=5, ACT Abs/Relu for s in {6,7}), then PE-rotated
    into j-layout once at startup (Pool evacuates).
"""

import os
import sys

import numpy as np

if "/opt/trn_rl_repo" not in sys.path:
    sys.path.insert(0, "/opt/trn_rl_repo")

# Problem constants (hardcoded per harness contract).
B, C, O, H, W = 8, 256, 8, 64, 64
G = 8
CPG = C // G          # 32 channels per group
HW = H * W            # 4096
NCORES = 8
NCHUNK = 16           # hw chunks on partitions: p = pos*16 + hw//256
U = HW // NCHUNK      # 256 hw elements per partition
CP = 2                # channels per group per pass
NCPASS = CPG // CP    # 16 passes
GH = G // 2           # groups per PE/PSUM half
# Static hat centers valid for |offset| < OFF_BOUND (13 hats total).
OFF_BOUND = 5.999
STATIC_CENTERS = [[0], [1], [-6, 2], [-5, 3], [-4, 4], [-3, 5], [-2, 6], [-1]]
# Tuning knobs.
POOL_UNITS = int(os.environ.get("BASS_DEO_POOL_UNITS", "5"))
USPLIT = int(os.environ.get("BASS_DEO_USPLIT", "0"))
NACT_PLANES = int(os.environ.get("BASS_DEO_ACT_PLANES", "1"))

_PROGRAM_CACHE = {}


def _centers_for_bound(maxa: float):
    kmax = int(maxa) // 8 + 2
    out = []
    for s in range(O):
        cs = [s + 8 * k for k in range(-kmax, kmax + 1)
              if (s + 8 * k - 1 < maxa) and (s + 8 * k + 1 > -maxa)]
        out.append(cs)
    return out


def _build_program(centers):
    import concourse.bass as bass  # noqa: F401
    import concourse.tile as tile
    from concourse import bacc, mybir

    f32 = mybir.dt.float32
    f16 = mybir.dt.float16
    Alu = mybir.AluOpType
    Act = mybir.ActivationFunctionType

    terms = [s for s in range(O) if centers[s]]
    assert terms, "no contributing terms"
    fuse_s0 = 0 in terms
    pe_terms = [s for s in terms if s != 0] if fuse_s0 else list(terms)

    # (term, g-half) product units; Pool owns the tail of the term list.
    nunits = 2 * len(terms)
    pool_units = min(POOL_UNITS, nunits - 2)
    tail = [(s, h) for s in reversed(terms) for h in (1, 0)]
    pool_set = set(tail[:pool_units])

    # Hat planes: the last NACT_PLANES terms are built on ACT, rest on DVE.
    act_planes = set(terms[len(terms) - min(NACT_PLANES, len(terms)):])

    nc = bacc.Bacc("TRN2", target_bir_lowering=False, debug=False)
    x_d = nc.declare_dram_parameter("x", [C, O, HW], f32, isOutput=False)
    off_d = nc.declare_dram_parameter("offset", [G, O, HW], f32, isOutput=False)
    out_d = nc.declare_dram_parameter("out", [C, O, HW], f32, isOutput=True)

    # DRAM views: iteration ((pos, chunk) -> 128 partitions, u). The pos
    # and chunk dims merge (pos stride 4096 = 16 chunks * 256), giving
    # 2-dim APs [[256, 128], [1, 256]] with 1KB payloads.
    x_r = x_d[:].rearrange("(g c) j (k u) -> g (j k) c u", g=G, k=NCHUNK)
    off_r = off_d[:].rearrange("g o (k u) -> g (o k) u", k=NCHUNK)
    out_r = out_d[:].rearrange("(g c) o (k u) -> g (o k) c u", g=G, k=NCHUNK)

    with tile.TileContext(nc) as tc:
        with (
            tc.tile_pool(name="setup", bufs=1) as setupp,
            tc.tile_pool(name="coefp", bufs=1) as coefp,
            tc.tile_pool(name="hatp", bufs=1) as hatp,
            tc.tile_pool(name="xp", bufs=3) as xp,
            tc.tile_pool(name="xhp", bufs=3) as xhp,
            tc.tile_pool(name="tdp", bufs=5) as tdp,
            tc.tile_pool(name="tpp", bufs=3) as tpp,
            tc.tile_pool(name="evp", bufs=2) as evp,
            tc.tile_pool(name="pscp", bufs=2) as pscp,
            tc.psum_pool(name="psp", bufs=1) as psp,
        ):
            # ---- offset + first x loads (offs split over 2 queues) ------
            offs32 = setupp.tile([128, G, U], f32)
            for g in range(G):
                eng = nc.sync if g % 2 == 0 else nc.scalar
                eng.dma_start(out=offs32[:, g], in_=off_r[g])

            def load_pass(pi):
                # channel-pair loads: the DRAM AP stays 3-dim with the
                # (pos, chunk) partition dim leading and c in the middle.
                c0 = pi * CP
                xf = xp.tile([128, G, CP, U], f32, tag="xf")
                for g in range(G):
                    nc.sync.dma_start(out=xf[:, g],
                                      in_=x_r[g, :, c0:c0 + CP])
                return xf

            xfs = {0: load_pass(0), 1: load_pass(1)}
            if NCPASS > 2:
                xfs[2] = load_pass(2)

            # ---- Pool: signed permutation matrices ----------------------
            # P[sign][s][pi, po] = +-1 iff po = (pi - 16s) mod 128.
            # Coefficient planes are stored NEGATED (saves a hat instr);
            # the minus family restores the sign in the pass matmuls.
            iot = setupp.tile([128, 128], f16)
            nc.gpsimd.iota(iot[:], pattern=[[1, 128]], base=0,
                           channel_multiplier=-1,
                           allow_small_or_imprecise_dtypes=True)
            P = setupp.tile([128, 2, O, 128], f16)  # [sign(0:+,1:-), s]
            nc.gpsimd.memset(P[:], 0.0)
            # minus: pass matmuls (neg planes) + perms of ACT (pos) planes
            minus_needed = set(pe_terms) | {(O - s) % O
                                            for s in act_planes if s}
            plus_needed = {(O - s) % O for s in terms
                           if s and s not in act_planes}
            ptmp = setupp.tile([128, 128], f16)
            for s in sorted(plus_needed | minus_needed):
                ns = 16 * s
                # iot = i - p; diagonal at i-p = -ns, wrap at 128-ns.
                nc.gpsimd.tensor_scalar(
                    out=P[:, 0, s], in0=iot[:], scalar1=float(-ns),
                    scalar2=None, op0=Alu.is_equal)
                if ns > 0:
                    nc.gpsimd.tensor_scalar(
                        out=ptmp[:], in0=iot[:], scalar1=float(128 - ns),
                        scalar2=None, op0=Alu.is_equal)
                    nc.gpsimd.tensor_tensor(
                        out=P[:, 0, s], in0=P[:, 0, s],
                        in1=ptmp[:], op=Alu.add)
            if minus_needed:
                nc.gpsimd.tensor_scalar(
                    out=P[:, 1], in0=P[:, 0], scalar1=-1.0,
                    scalar2=None, op0=Alu.mult)

            # Bias columns for the ACT Abs stages of every hat plane:
            # 1-center plane s needs -c; 2-center (8-apart) uses the nested
            # fold ||x-(c+4)|-4| and needs -(c+4) and the shared -4.
            act_cens = set()
            for s in terms:
                cs = centers[s]
                if s not in act_planes and len(cs) == 2 \
                        and cs[1] - cs[0] == 8:
                    act_cens.add(-(cs[0] + 4))
                    act_cens.add(-4)
                elif s not in act_planes:
                    act_cens.update(-c for c in cs)
                else:
                    act_cens.update(-c for c in cs)
            act_cens = sorted(act_cens)
            cen_col = {c: i for i, c in enumerate(act_cens)}
            bias_t = setupp.tile([128, max(len(act_cens), 1)], f32)
            for c, i in cen_col.items():
                nc.gpsimd.memset(bias_t[:, i:i + 1], float(c))

            # ---- coefficient planes -------------------------------------
            # coef_j[:, s]: C_s pre-rotated into j-partition layout.
            coef_j = coefp.tile([128, O, G, U], f16)

            def bcol(v):
                i = cen_col[v]
                return bias_t[:, i:i + 1]

            def emit_z(s, ztag):
                # z = distance to the nearest hat center, via ACT Abs.
                # Two 8-apart centers fold: min(|x-c|,|x-c-8|)=||x-c-4|-4|.
                cs = centers[s]
                z = hatp.tile([128, G, U], f16, tag=ztag, name=ztag)
                if len(cs) == 2 and cs[1] - cs[0] == 8:
                    nc.scalar.activation(out=z[:], in_=offs32[:],
                                         func=Act.Abs,
                                         bias=bcol(-(cs[0] + 4)), scale=1.0)
                    nc.scalar.activation(out=z[:], in_=z[:], func=Act.Abs,
                                         bias=bcol(-4), scale=1.0)
                else:
                    nc.scalar.activation(out=z[:], in_=offs32[:],
                                         func=Act.Abs,
                                         bias=bcol(-cs[0]), scale=1.0)
                    for cen in cs[1:]:
                        z2 = hatp.tile([128, G, U], f16, tag="qn0")
                        nc.scalar.activation(out=z2[:], in_=offs32[:],
                                             func=Act.Abs,
                                             bias=bcol(-cen), scale=1.0)
                        nc.gpsimd.tensor_tensor(
                            out=z[:], in0=z[:], in1=z2[:], op=Alu.min)
                return z

            def hats_dve(s, dest):
                # NEGATED plane: -C_s = min(1, z) - 1.  ACT computes z,
                # DVE finishes with one (min, add) tensor_scalar.
                z = emit_z(s, f"z{s % 2}")
                nc.vector.tensor_scalar(
                    out=dest, in0=z[:], scalar1=1.0,
                    scalar2=-1.0, op0=Alu.min, op1=Alu.add)

            def hats_act(s, dest):
                # POSITIVE plane on ACT: C_s = relu(1 - z).
                z = emit_z(s, f"z{s % 2}")
                nc.scalar.activation(out=dest, in_=z[:], func=Act.Relu,
                                     bias=1.0, scale=-1.0)

            def build_plane(s, psum_tag=None):
                if s == 0:
                    # identity rotation: write straight into coef_j
                    hats_dve(0, coef_j[:, 0])
                    return
                co = hatp.tile([128, G, U], f16, tag="co", name="co")
                act_built = s in act_planes
                (hats_act if act_built else hats_dve)(s, co[:])
                # PE-rotate o-layout -> j-layout with P_{(8-s)%8}; ACT
                # planes are positive, so their perm also negates.
                w = P[:, 1 if act_built else 0, (O - s) % O, :]
                ptag = psum_tag or f"ps{s % 2}"
                pst = psp.tile([128, GH, CP, U], f32, tag=ptag, name=ptag)
                cof = co[:].rearrange("p g u -> p (g u)")
                for c4 in range(4):
                    nc.tensor.matmul(
                        pst[:, c4], w,
                        cof[:, 512 * c4:512 * (c4 + 1)],
                        start=True, stop=True)
                # ACT evacuates (GPSIMD cannot access PSUM on real HW).
                nc.scalar.copy(
                    out=coef_j[:, s].rearrange("p g u -> p (g u)"),
                    in_=pst[:].rearrange("p a c u -> p (a c u)"))

            # Build order: first 3 DVE planes up front; the rest interleave
            # with pass-0 DVE muls (positional queues: this keeps DVE's
            # plane->perm->evac pipeline just ahead of its consumers).
            # ACT planes go after the first xh convert so xh0 lands early.
            act_plane_list = [s for s in terms if s in act_planes]
            dve_built = [s for s in terms if s not in act_planes]
            # Planes consumed by pass-0's first-half Pool units come first,
            # then the DVE-consumed planes (a permuted one leading so the
            # PE/ACT perm-evac pipeline starts early); the tail interleaves
            # with pass-0 DVE muls.
            pool_h0 = [s for s in dve_built if (s, 0) in pool_set]
            rest = [s for s in dve_built if s not in pool_h0]
            if len(rest) > 1 and rest[0] == 0:
                rest[0], rest[1] = rest[1], rest[0]
            dve_plane_list = pool_h0 + rest

            def convert_pass(xf):
                xh = xhp.tile([128, G, CP, U], f16, tag="xh")
                nc.scalar.copy(out=xh[:], in_=xf[:])
                return xh

            # xh0's convert goes ahead of the plane evacuations in ACT's
            # queue: it gates every pass-0 product.
            xhs = {0: convert_pass(xfs[0])}
            for s in dve_plane_list[:5]:
                build_plane(s)
            planes_pending = dve_plane_list[5:]
            if act_plane_list:
                build_plane(act_plane_list[0])
            if NCPASS > 1:
                xhs[1] = convert_pass(xfs[1])
            for s in act_plane_list[1:]:
                build_plane(s)

            def emit_mul(pool_eng, s, xh, gs, t, u0=0, u1=U):
                xin = xh[:, gs, :, u0:u1]
                cin = (coef_j[:, s, gs, u0:u1].unsqueeze(2)
                       .to_broadcast([128, GH, CP, u1 - u0]))
                tout = t[:, :, :, u0:u1]
                eng = nc.gpsimd if pool_eng else nc.vector
                eng.tensor_mul(out=tout, in0=xin, in1=cin)

            for pi in range(NCPASS):
                c0 = pi * CP
                # pass 0's ACT queue must reach the lazy coef-plane evacs
                # quickly: defer its prefetch convert until after the muls.
                if pi + 2 < NCPASS and pi > 0:
                    xhs[pi + 2] = convert_pass(xfs[pi + 2])
                if pi + 3 < NCPASS:
                    xfs[pi + 3] = load_pass(pi + 3)
                xh = xhs.pop(pi)
                xfs.pop(pi, None)

                ev = evp.tile([128, G, CP, U], f32, tag="ev")

                for h in range(2):
                    gs = slice(h * GH, (h + 1) * GH)
                    ps = psp.tile([128, GH, CP, U], f32, tag=f"ps{h}",
                                  name=f"ps{h}")
                    dve_units = [s for s in terms if (s, h) not in pool_set]
                    split_s = (0 if USPLIT > 0 and fuse_s0
                               and 0 in dve_units else None)
                    unit_tiles = {}
                    for s in terms:           # Pool units first
                        if (s, h) not in pool_set:
                            continue
                        if s in planes_pending:
                            planes_pending.remove(s)
                            build_plane(s, psum_tag=f"ps{1 - h}")
                        t = tpp.tile([128, GH, CP, U], f16, tag="tp",
                                     name="tp")
                        emit_mul(True, s, xh, gs, t)
                        unit_tiles[s] = t
                    if split_s is not None:
                        # u-split the s=0 unit: its tile feeds only Pool's
                        # own evac-add, so the Pool part adds no PE coupling
                        tsp = tdp.tile([128, GH, CP, U], f16, tag="td",
                                       name="td")
                        emit_mul(True, split_s, xh, gs, tsp, 0, USPLIT)
                        unit_tiles[split_s] = tsp
                    for s in dve_units:
                        if s == split_s:
                            emit_mul(False, s, xh, gs, unit_tiles[s],
                                     USPLIT, U)
                        else:
                            t = tdp.tile([128, GH, CP, U], f16, tag="td",
                                         name="td")
                            emit_mul(False, s, xh, gs, t)
                            unit_tiles[s] = t
                        if planes_pending:
                            # perms go through the not-yet-started other
                            # psum half to avoid a WAR cycle with this half
                            build_plane(planes_pending.pop(0),
                                        psum_tag=f"ps{1 - h}")

                    npe = len(pe_terms)
                    for k, s in enumerate(pe_terms):
                        t = unit_tiles[s]
                        for gg in range(GH):
                            nc.tensor.matmul(
                                ps[:, gg], P[:, 1, s, :], t[:, gg],
                                start=(k == 0), stop=(k == npe - 1))

                    # Evacuate: ACT copies PSUM -> fp16 scratch (GPSIMD has
                    # no PSUM access); Pool subtracts the unrotated s=0
                    # product (its plane is stored negated).
                    if fuse_s0:
                        psc = pscp.tile([128, GH, CP, U], f16, tag="psc",
                                        name="psc")
                        nc.scalar.copy(out=psc[:], in_=ps[:])
                        nc.gpsimd.tensor_tensor(
                            out=ev[:, gs], in0=psc[:], in1=unit_tiles[0][:],
                            op=Alu.subtract)
                    else:
                        nc.scalar.copy(out=ev[:, gs], in_=ps[:])

                    # Channel-pair stores for this half's groups right
                    # after its evacuation: h0 on SP, h1 on ACT (last
                    # pass: alternate queues to shorten the drain).
                    last = pi == NCPASS - 1
                    for i, g in enumerate(range(h * GH, (h + 1) * GH)):
                        if last:
                            seng = nc.sync if i % 2 else nc.scalar
                        else:
                            seng = nc.sync if h == 0 else nc.scalar
                        seng.dma_start(out=out_r[g, :, c0:c0 + CP],
                                       in_=ev[:, g])

                if pi == 0 and 2 < NCPASS:
                    xhs[2] = convert_pass(xfs[2])
    return nc


def _get_program(centers):
    key = tuple(tuple(c) for c in centers)
    prog = _PROGRAM_CACHE.get(key)
    if prog is None:
        prog = _build_program(centers)
        prog.finalize()
        _PROGRAM_CACHE[key] = prog
    return prog


_LAST_RESULTS = None  # BassKernelResults of the most recent kernel() call


def kernel(x: np.ndarray, offset: np.ndarray) -> np.ndarray:
    global _LAST_RESULTS
    from concourse.bass_utils import run_bass_kernel_spmd

    x = np.ascontiguousarray(np.asarray(x, dtype=np.float32))
    offset = np.ascontiguousarray(np.asarray(offset, dtype=np.float32))
    assert x.shape == (B, C, O, H, W) and offset.shape == (B, G, O, H, W)

    maxa = float(np.abs(offset).max())
    centers = (STATIC_CENTERS if maxa < OFF_BOUND
               else _centers_for_bound(maxa + 1e-3))
    nc = _get_program(centers)

    xs = x.reshape(B, C, O, HW)
    offs = offset.reshape(B, G, O, HW)
    in_maps = [{"x": xs[b], "offset": offs[b]} for b in range(NCORES)]
    trace = bool(int(os.environ.get("BASS_DEO_TRACE", "0")))
    kw = {}
    if trace:
        kw["trace"] = True
        tdir = os.environ.get("BASS_DEO_TRACE_DIR")
        if tdir:
            kw["tmpdir"] = tdir
    br = run_bass_kernel_spmd(nc, in_maps, list(range(NCORES)), **kw)
    _LAST_RESULTS = br
    out = np.stack([br.results[b]["out"] for b in range(NCORES)])
    return out.reshape(B, C, O, H, W)


if __name__ == "__main__":
    xs = np.load("/tmp/x.npy")
    offs = np.load("/tmp/off.npy")
    got = kernel(xs, offs)
    exp = np.load("/tmp/expected.npy")
    d = np.abs(got - exp)
    print("absmax:", d.max(), "rel:", d.max() / np.abs(exp).max())


# revision 82
# speedup vs baseline: 2.8388x; 1.0174x over previous
"""Trainium2 Bass kernel for deformable orientation sampling (DeoLayer).

Math:
  out[b,c,o,h,w] = (1-w1)*x[b,c,i0,h,w] + w1*x[b,c,i1,h,w]
  where p = o + offset[b,g(c),o,h,w], i0 = floor(p) mod O, i1 = (i0+1) mod O,
  w1 = frac(offset), O = 8 orientations, G = 8 groups (32 channels each).

Reformulated as a dense 8-term cyclic weighted sum with "periodized hat"
coefficients (non-contributing terms are exactly 0):
  out[...,o,hw] = sum_{s=0..7} C_s[g,o,hw] * x[...,(o+s)%8,hw]
  C_s = sum_k relu(1 - |offset - (s + 8k)|)   (hats have disjoint support)

Distribution: pure data parallel, batch b -> core b (8 cores, no comms).

Schedule (built for CoreSim's v1 cost model: each engine is one serial
timeline that also executes its own DMAs; a DMA costs the out-AP's
after-first-dim bytes * 0.386ns/B (2x under 512B runs, 500ns floor); Pool
runs tensor ops at 0.833ns/elem; DVE gets 2x on fp16 tensor_tensor and 4x
on fp16 tensor_scalar; PE matmuls cost out-free-rows * 0.42ns for fp16
moving data, with free weight switches; cross-engine syncs are positional
per-engine counters, so per-engine emission order is execution order):

  * SBUF partition p = pos*16 + hw//256, u = hw%256, where pos is the
    orientation (j on inputs, o on outputs).  The (pos, hw-chunk) dims
    merge with u into a single 128-long stride-256 DRAM dim, so every
    x/out transfer is a [128, 1KB] DMA at the 500ns floor: 16 loads + 16
    stores per 2-channel pass, split across the SP and ACT queues.
  * All on-chip math fp16 (tolerance 2e-2, actual err ~2e-3).
  * The cyclic roll j -> o=(j-s)%8 is a partition-block rotation done by
    the otherwise-idle PE: per term a permutation-matrix matmul
    accumulates the product tile into PSUM (fp32).  s=0 needs no
    rotation: Pool adds its product during PSUM evacuation.
  * Products t_s = C_s * x as 16 (term, group-half) units, one
    instruction each (no roll slicing), split DVE / Pool (stride-0
    c-broadcast of the group-shared C_s).
  * Coefficient planes are built in o-partition layout (DVE tensor_scalar
    hats at 4x for s<=5, ACT Abs/Relu for s in {6,7}), then PE-rotated
    into j-layout once at startup (Pool evacuates).
"""

import os
import sys

import numpy as np

if "/opt/trn_rl_repo" not in sys.path:
    sys.path.insert(0, "/opt/trn_rl_repo")

# Problem constants (hardcoded per harness contract).
B, C, O, H, W = 8, 256, 8, 64, 64
G = 8
CPG = C // G          # 32 channels per group
HW = H * W            # 4096
NCORES = 8
NCHUNK = 16           # hw chunks on partitions: p = pos*16 + hw//256
U = HW // NCHUNK      # 256 hw elements per partition
CP = 2                # channels per group per pass
NCPASS = CPG // CP    # 16 passes
GH = G // 2           # groups per PE/PSUM half
# Static hat centers valid for |offset| < OFF_BOUND (13 hats total).
OFF_BOUND = 5.999
STATIC_CENTERS = [[0], [1], [-6, 2], [-5, 3], [-4, 4], [-3, 5], [-2, 6], [-1]]
# Tuning knobs.
POOL_UNITS = int(os.environ.get("BASS_DEO_POOL_UNITS", "5"))
USPLIT = int(os.environ.get("BASS_DEO_USPLIT", "0"))
NACT_PLANES = int(os.environ.get("BASS_DEO_ACT_PLANES", "0"))
NSELF_PLANES = int(os.environ.get("BASS_DEO_SELF_PLANES", "0"))

_PROGRAM_CACHE = {}


def _centers_for_bound(maxa: float):
    kmax = int(maxa) // 8 + 2
    out = []
    for s in range(O):
        cs = [s + 8 * k for k in range(-kmax, kmax + 1)
              if (s + 8 * k - 1 < maxa) and (s + 8 * k + 1 > -maxa)]
        out.append(cs)
    return out


def _build_program(centers):
    import concourse.bass as bass  # noqa: F401
    import concourse.tile as tile
    from concourse import bacc, mybir

    f32 = mybir.dt.float32
    f16 = mybir.dt.float16
    Alu = mybir.AluOpType
    Act = mybir.ActivationFunctionType

    terms = [s for s in range(O) if centers[s]]
    assert terms, "no contributing terms"
    fuse_s0 = 0 in terms
    pe_terms = [s for s in terms if s != 0] if fuse_s0 else list(terms)

    # (term, g-half) product units; Pool owns the tail of the term list.
    nunits = 2 * len(terms)
    pool_units = min(POOL_UNITS, nunits - 2)
    tail = [(s, h) for s in reversed(terms) for h in (1, 0)]
    pool_set = set(tail[:pool_units])

    # Hat planes: the last NACT_PLANES terms are built on ACT, rest get
    # their z from ACT Abs with a DVE final; the first NSELF planes are
    # built entirely on DVE (idle at startup) to shorten ACT's queue.
    act_planes = (set(terms[len(terms) - NACT_PLANES:])
                  if NACT_PLANES > 0 else set())
    nself = min(NSELF_PLANES, len(terms))
    self_planes = set(s for s in terms if s not in act_planes)
    self_planes = set(sorted(self_planes)[:nself])

    nc = bacc.Bacc("TRN2", target_bir_lowering=False, debug=False)
    x_d = nc.declare_dram_parameter("x", [C, O, HW], f32, isOutput=False)
    off_d = nc.declare_dram_parameter("offset", [G, O, HW], f32, isOutput=False)
    out_d = nc.declare_dram_parameter("out", [C, O, HW], f32, isOutput=True)

    # DRAM views: iteration ((pos, chunk) -> 128 partitions, u). The pos
    # and chunk dims merge (pos stride 4096 = 16 chunks * 256), giving
    # 2-dim APs [[256, 128], [1, 256]] with 1KB payloads.
    x_r = x_d[:].rearrange("(g c) j (k u) -> g (j k) c u", g=G, k=NCHUNK)
    off_r = off_d[:].rearrange("g o (k u) -> g (o k) u", k=NCHUNK)
    out_r = out_d[:].rearrange("(g c) o (k u) -> g (o k) c u", g=G, k=NCHUNK)

    with tile.TileContext(nc) as tc:
        with (
            tc.tile_pool(name="setup", bufs=1) as setupp,
            tc.tile_pool(name="coefp", bufs=1) as coefp,
            tc.tile_pool(name="hatp", bufs=1) as hatp,
            tc.tile_pool(name="xp", bufs=3) as xp,
            tc.tile_pool(name="xhp", bufs=3) as xhp,
            tc.tile_pool(name="tdp", bufs=5) as tdp,
            tc.tile_pool(name="tpp", bufs=3) as tpp,
            tc.tile_pool(name="evp", bufs=2) as evp,
            tc.tile_pool(name="pscp", bufs=2) as pscp,
            tc.psum_pool(name="psp", bufs=1) as psp,
        ):
            # ---- offset + first x loads (offs split over 2 queues) ------
            offs32 = setupp.tile([128, G, U], f32)
            for g in range(G):
                eng = nc.sync if g % 2 == 0 else nc.scalar
                eng.dma_start(out=offs32[:, g], in_=off_r[g])

            def load_pass(pi):
                # channel-pair loads: the DRAM AP stays 3-dim with the
                # (pos, chunk) partition dim leading and c in the middle.
                c0 = pi * CP
                xf = xp.tile([128, G, CP, U], f32, tag="xf")
                for g in range(G):
                    nc.sync.dma_start(out=xf[:, g],
                                      in_=x_r[g, :, c0:c0 + CP])
                return xf

            xfs = {0: load_pass(0), 1: load_pass(1)}
            if NCPASS > 2:
                xfs[2] = load_pass(2)

            # fp16 offsets: feeds the DVE-built startup planes.
            if self_planes:
                offs = setupp.tile([128, G, U], f16)
                nc.scalar.copy(out=offs[:], in_=offs32[:])

            # ---- Pool: signed permutation matrices ----------------------
            # P[sign][s][pi, po] = +-1 iff po = (pi - 16s) mod 128.
            # Coefficient planes are stored NEGATED (saves a hat instr);
            # the minus family restores the sign in the pass matmuls.
            iot = setupp.tile([128, 128], f16)
            nc.gpsimd.iota(iot[:], pattern=[[1, 128]], base=0,
                           channel_multiplier=-1,
                           allow_small_or_imprecise_dtypes=True)
            P = setupp.tile([128, 2, O, 128], f16)  # [sign(0:+,1:-), s]
            nc.gpsimd.memset(P[:], 0.0)
            # minus: pass matmuls (neg planes) + perms of ACT (pos) planes
            minus_needed = set(pe_terms) | {(O - s) % O
                                            for s in act_planes if s}
            plus_needed = {(O - s) % O for s in terms
                           if s and s not in act_planes}
            ptmp = setupp.tile([128, 128], f16)
            for s in sorted(plus_needed | minus_needed):
                ns = 16 * s
                # iot = i - p; diagonal at i-p = -ns, wrap at 128-ns.
                nc.gpsimd.tensor_scalar(
                    out=P[:, 0, s], in0=iot[:], scalar1=float(-ns),
                    scalar2=None, op0=Alu.is_equal)
                if ns > 0:
                    nc.gpsimd.tensor_scalar(
                        out=ptmp[:], in0=iot[:], scalar1=float(128 - ns),
                        scalar2=None, op0=Alu.is_equal)
                    nc.gpsimd.tensor_tensor(
                        out=P[:, 0, s], in0=P[:, 0, s],
                        in1=ptmp[:], op=Alu.add)
            if minus_needed:
                nc.gpsimd.tensor_scalar(
                    out=P[:, 1], in0=P[:, 0], scalar1=-1.0,
                    scalar2=None, op0=Alu.mult)

            # Bias columns for the ACT Abs stages of every hat plane:
            # 1-center plane s needs -c; 2-center (8-apart) uses the nested
            # fold ||x-(c+4)|-4| and needs -(c+4) and the shared -4.
            act_cens = set()
            for s in terms:
                cs = centers[s]
                if len(cs) == 2 and cs[1] - cs[0] == 8:
                    act_cens.add(-(cs[0] + 4))
                    act_cens.add(-4)
                else:
                    act_cens.update(-c for c in cs)
            act_cens = sorted(act_cens)
            cen_col = {c: i for i, c in enumerate(act_cens)}
            bias_t = setupp.tile([128, max(len(act_cens), 1)], f32)
            for c, i in cen_col.items():
                nc.gpsimd.memset(bias_t[:, i:i + 1], float(c))

            # ---- coefficient planes -------------------------------------
            # coef_j[:, s]: C_s pre-rotated into j-partition layout.
            coef_j = coefp.tile([128, O, G, U], f16)

            def bcol(v):
                i = cen_col[v]
                return bias_t[:, i:i + 1]

            def emit_z(s, ztag):
                # z = distance to the nearest hat center, via ACT Abs.
                # Two 8-apart centers fold: min(|x-c|,|x-c-8|)=||x-c-4|-4|.
                cs = centers[s]
                z = hatp.tile([128, G, U], f16, tag=ztag, name=ztag)
                if len(cs) == 2 and cs[1] - cs[0] == 8:
                    nc.scalar.activation(out=z[:], in_=offs32[:],
                                         func=Act.Abs,
                                         bias=bcol(-(cs[0] + 4)), scale=1.0)
                    nc.scalar.activation(out=z[:], in_=z[:], func=Act.Abs,
                                         bias=bcol(-4), scale=1.0)
                else:
                    nc.scalar.activation(out=z[:], in_=offs32[:],
                                         func=Act.Abs,
                                         bias=bcol(-cs[0]), scale=1.0)
                    for cen in cs[1:]:
                        z2 = hatp.tile([128, G, U], f16, tag="qn0")
                        nc.scalar.activation(out=z2[:], in_=offs32[:],
                                             func=Act.Abs,
                                             bias=bcol(-cen), scale=1.0)
                        nc.gpsimd.tensor_tensor(
                            out=z[:], in0=z[:], in1=z2[:], op=Alu.min)
                return z

            def hats_dve(s, dest):
                # NEGATED plane: -C_s = min(1, z) - 1.  ACT computes z,
                # DVE finishes with one (min, add) tensor_scalar.
                z = emit_z(s, f"z{s % 2}")
                nc.vector.tensor_scalar(
                    out=dest, in0=z[:], scalar1=1.0,
                    scalar2=-1.0, op0=Alu.min, op1=Alu.add)

            def abs_dve(src_c, dtag):
                # |offs - c| via max(offs-c, c-offs): plain ts/tt ops only.
                d = hatp.tile([128, G, U], f16, tag=dtag, name=dtag)
                dn = hatp.tile([128, G, U], f16, tag=dtag + "n",
                               name=dtag + "n")
                nc.vector.tensor_scalar(
                    out=d[:], in0=offs[:], scalar1=float(src_c),
                    scalar2=None, op0=Alu.subtract)
                nc.vector.tensor_scalar(
                    out=dn[:], in0=offs[:], scalar1=-1.0,
                    scalar2=float(src_c), op0=Alu.mult, op1=Alu.add)
                nc.vector.tensor_tensor(out=d[:], in0=d[:], in1=dn[:],
                                        op=Alu.max)
                return d

            def hats_dve_self(s, dest):
                # NEGATED plane entirely on DVE (startup latency path).
                cs = centers[s]
                if len(cs) == 2 and cs[1] - cs[0] == 8:
                    # min(|x-c|,|x-c-8|) = ||x-c-4|-4|
                    ay = abs_dve(cs[0] + 4, "z0")
                    e = hatp.tile([128, G, U], f16, tag="qn0", name="qn0")
                    en = hatp.tile([128, G, U], f16, tag="z1", name="z1")
                    nc.vector.tensor_scalar(
                        out=e[:], in0=ay[:], scalar1=4.0,
                        scalar2=None, op0=Alu.subtract)
                    nc.vector.tensor_scalar(
                        out=en[:], in0=ay[:], scalar1=-1.0,
                        scalar2=4.0, op0=Alu.mult, op1=Alu.add)
                    nc.vector.tensor_tensor(out=e[:], in0=e[:], in1=en[:],
                                            op=Alu.max)
                    z = e
                else:
                    z = abs_dve(cs[0], "z0")
                    for cen in cs[1:]:
                        z2 = abs_dve(cen, "qn0")
                        nc.vector.tensor_tensor(
                            out=z[:], in0=z[:], in1=z2[:], op=Alu.min)
                nc.vector.tensor_scalar(
                    out=dest, in0=z[:], scalar1=1.0,
                    scalar2=-1.0, op0=Alu.min, op1=Alu.add)

            def hats_act(s, dest):
                # POSITIVE plane on ACT: C_s = relu(1 - z).
                z = emit_z(s, f"z{s % 2}")
                nc.scalar.activation(out=dest, in_=z[:], func=Act.Relu,
                                     bias=1.0, scale=-1.0)

            def build_plane(s, psum_tag=None):
                if s in self_planes:
                    hat_fn = hats_dve_self
                elif s in act_planes:
                    hat_fn = hats_act
                else:
                    hat_fn = hats_dve
                if s == 0:
                    # identity rotation: write straight into coef_j
                    hat_fn(0, coef_j[:, 0])
                    return
                co = hatp.tile([128, G, U], f16, tag="co", name="co")
                act_built = s in act_planes
                hat_fn(s, co[:])
                # PE-rotate o-layout -> j-layout with P_{(8-s)%8}; ACT
                # planes are positive, so their perm also negates.
                w = P[:, 1 if act_built else 0, (O - s) % O, :]
                ptag = psum_tag or f"ps{s % 2}"
                pst = psp.tile([128, GH, CP, U], f32, tag=ptag, name=ptag)
                cof = co[:].rearrange("p g u -> p (g u)")
                for c4 in range(4):
                    nc.tensor.matmul(
                        pst[:, c4], w,
                        cof[:, 512 * c4:512 * (c4 + 1)],
                        start=True, stop=True)
                # ACT evacuates (GPSIMD cannot access PSUM on real HW).
                nc.scalar.copy(
                    out=coef_j[:, s].rearrange("p g u -> p (g u)"),
                    in_=pst[:].rearrange("p a c u -> p (a c u)"))

            # Build order: first 3 DVE planes up front; the rest interleave
            # with pass-0 DVE muls (positional queues: this keeps DVE's
            # plane->perm->evac pipeline just ahead of its consumers).
            # ACT planes go after the first xh convert so xh0 lands early.
            act_plane_list = [s for s in terms if s in act_planes]
            dve_built = [s for s in terms if s not in act_planes]
            # Planes consumed by pass-0's first-half Pool units come first,
            # then the DVE-consumed planes (a permuted one leading so the
            # PE/ACT perm-evac pipeline starts early); the tail interleaves
            # with pass-0 DVE muls.
            pool_h0 = [s for s in dve_built if (s, 0) in pool_set]
            rest = [s for s in dve_built if s not in pool_h0]
            if len(rest) > 1 and rest[0] == 0:
                rest[0], rest[1] = rest[1], rest[0]
            dve_plane_list = pool_h0 + rest

            def convert_pass(xf):
                xh = xhp.tile([128, G, CP, U], f16, tag="xh")
                nc.scalar.copy(out=xh[:], in_=xf[:])
                return xh

            # xh0's convert goes ahead of the plane evacuations in ACT's
            # queue: it gates every pass-0 product.
            xhs = {0: convert_pass(xfs[0])}
            for s in dve_plane_list[:5]:
                build_plane(s)
            planes_pending = dve_plane_list[5:]
            if act_plane_list:
                build_plane(act_plane_list[0])
            if NCPASS > 1:
                xhs[1] = convert_pass(xfs[1])
            for s in act_plane_list[1:]:
                build_plane(s)

            def emit_mul(pool_eng, s, xh, gs, t, u0=0, u1=U):
                xin = xh[:, gs, :, u0:u1]
                cin = (coef_j[:, s, gs, u0:u1].unsqueeze(2)
                       .to_broadcast([128, GH, CP, u1 - u0]))
                tout = t[:, :, :, u0:u1]
                eng = nc.gpsimd if pool_eng else nc.vector
                eng.tensor_mul(out=tout, in0=xin, in1=cin)

            for pi in range(NCPASS):
                c0 = pi * CP
                # pass 0's ACT queue must reach the lazy coef-plane evacs
                # quickly: defer its prefetch convert until after the muls.
                if pi + 2 < NCPASS and pi > 0:
                    xhs[pi + 2] = convert_pass(xfs[pi + 2])
                if pi + 3 < NCPASS:
                    xfs[pi + 3] = load_pass(pi + 3)
                xh = xhs.pop(pi)
                xfs.pop(pi, None)

                ev = evp.tile([128, G, CP, U], f32, tag="ev")

                for h in range(2):
                    gs = slice(h * GH, (h + 1) * GH)
                    ps = psp.tile([128, GH, CP, U], f32, tag=f"ps{h}",
                                  name=f"ps{h}")
                    dve_units = [s for s in terms if (s, h) not in pool_set]
                    split_s = (0 if USPLIT > 0 and fuse_s0
                               and 0 in dve_units else None)
                    unit_tiles = {}
                    for s in terms:           # Pool units first
                        if (s, h) not in pool_set:
                            continue
                        if s in planes_pending:
                            planes_pending.remove(s)
                            build_plane(s, psum_tag=f"ps{1 - h}")
                        t = tpp.tile([128, GH, CP, U], f16, tag="tp",
                                     name="tp")
                        emit_mul(True, s, xh, gs, t)
                        unit_tiles[s] = t
                    if split_s is not None:
                        # u-split the s=0 unit: its tile feeds only Pool's
                        # own evac-add, so the Pool part adds no PE coupling
                        tsp = tdp.tile([128, GH, CP, U], f16, tag="td",
                                       name="td")
                        emit_mul(True, split_s, xh, gs, tsp, 0, USPLIT)
                        unit_tiles[split_s] = tsp
                    for s in dve_units:
                        if s == split_s:
                            emit_mul(False, s, xh, gs, unit_tiles[s],
                                     USPLIT, U)
                        else:
                            t = tdp.tile([128, GH, CP, U], f16, tag="td",
                                         name="td")
                            emit_mul(False, s, xh, gs, t)
                            unit_tiles[s] = t
                        if planes_pending:
                            # perms go through the not-yet-started other
                            # psum half to avoid a WAR cycle with this half
                            build_plane(planes_pending.pop(0),
                                        psum_tag=f"ps{1 - h}")

                    npe = len(pe_terms)
                    for k, s in enumerate(pe_terms):
                        t = unit_tiles[s]
                        for gg in range(GH):
                            nc.tensor.matmul(
                                ps[:, gg], P[:, 1, s, :], t[:, gg],
                                start=(k == 0), stop=(k == npe - 1))

                    # Evacuate: ACT copies PSUM -> fp16 scratch (GPSIMD has
                    # no PSUM access); Pool combines the unrotated s=0
                    # product (subtract when its plane is stored negated).
                    if fuse_s0:
                        psc = pscp.tile([128, GH, CP, U], f16, tag="psc",
                                        name="psc")
                        nc.scalar.copy(out=psc[:], in_=ps[:])
                        s0_op = (Alu.subtract if 0 not in act_planes
                                 else Alu.add)
                        nc.gpsimd.tensor_tensor(
                            out=ev[:, gs], in0=psc[:], in1=unit_tiles[0][:],
                            op=s0_op)
                    else:
                        nc.scalar.copy(out=ev[:, gs], in_=ps[:])

                    # Channel-pair stores for this half's groups right
                    # after its evacuation, on SP (last pass: alternate
                    # queues to shorten the drain).
                    last = pi == NCPASS - 1
                    for i, g in enumerate(range(h * GH, (h + 1) * GH)):
                        seng = nc.scalar if (last and i % 2) else nc.sync
                        seng.dma_start(out=out_r[g, :, c0:c0 + CP],
                                       in_=ev[:, g])

                if pi == 0 and 2 < NCPASS:
                    xhs[2] = convert_pass(xfs[2])
    return nc


def _get_program(centers):
    key = tuple(tuple(c) for c in centers)
    prog = _PROGRAM_CACHE.get(key)
    if prog is None:
        prog = _build_program(centers)
        prog.finalize()
        _PROGRAM_CACHE[key] = prog
    return prog


_LAST_RESULTS = None  # BassKernelResults of the most recent kernel() call


def kernel(x: np.ndarray, offset: np.ndarray) -> np.ndarray:
    global _LAST_RESULTS
    from concourse.bass_utils import run_bass_kernel_spmd

    x = np.ascontiguousarray(np.asarray(x, dtype=np.float32))
    offset = np.ascontiguousarray(np.asarray(offset, dtype=np.float32))
    assert x.shape == (B, C, O, H, W) and offset.shape == (B, G, O, H, W)

    maxa = float(np.abs(offset).max())
    centers = (STATIC_CENTERS if maxa < OFF_BOUND
               else _centers_for_bound(maxa + 1e-3))
    nc = _get_program(centers)

    xs = x.reshape(B, C, O, HW)
    offs = offset.reshape(B, G, O, HW)
    in_maps = [{"x": xs[b], "offset": offs[b]} for b in range(NCORES)]
    trace = bool(int(os.environ.get("BASS_DEO_TRACE", "0")))
    kw = {}
    if trace:
        kw["trace"] = True
        tdir = os.environ.get("BASS_DEO_TRACE_DIR")
        if tdir:
            kw["tmpdir"] = tdir
    br = run_bass_kernel_spmd(nc, in_maps, list(range(NCORES)), **kw)
    _LAST_RESULTS = br
    out = np.stack([br.results[b]["out"] for b in range(NCORES)])
    return out.reshape(B, C, O, H, W)


if __name__ == "__main__":
    xs = np.load("/tmp/x.npy")
    offs = np.load("/tmp/off.npy")
    got = kernel(xs, offs)
    exp = np.load("/tmp/expected.npy")
    d = np.abs(got - exp)
    print("absmax:", d.max(), "rel:", d.max() / np.abs(exp).max())
